# revision 16
# baseline (speedup 1.0000x reference)
"""Multi-head attention (B=2, S=2048, D=1024, H=16, causal, interleaved RoPE)
on 8 Trainium2 NeuronCores.

Sharding: tensor-parallel over heads - 2 heads (128 channels) per core.
Each core computes its Q/K/V projections, RoPE, causal attention, and a
row-parallel partial of the output projection; the host sums the bf16
partials in fp32.

All matmuls in bf16 with fp32 PSUM accumulation. Key structure:
  * x^T is pre-transposed and cast to bf16 on the host (block-major layout)
    so no on-device transposes are needed for the q/k projections.
  * Q/K projection weights are host-permuted so each head's dims are
    [evens(32), odds(32)]; the RoPE pair-swap is then a 32-partition-block
    permutation done with ONE PE matmul against a 0/1 permutation matrix.
  * V is projected directly in transposed layout (lhsT = x^T block, rhs =
    Wv^T) as N=128 matmuls, so v_sb[k-pos, ch] needs no PE transposes and
    the psum->sbuf evacuation is a single contiguous copy fused with the
    bias add.
  * Attention uses the S^T layout: scores psum [k(128part), q(512)] via
    matmul(lhsT=K^T, rhs=Q^T); the two heads run CONCURRENTLY on disjoint
    PE row groups (K=64 each, auto tile_position from base partition),
    writing the two banks of one [128,1024] psum tile; ONE exp over both
    heads; causal masking multiplies only the [128,2x128] diagonal strip
    by a triangular constant (DVE). PV via matmul(lhsT=V_aug, rhs=P^T)
    with V_aug = [ones | 63 zero-pad | v dims] per head: the softmax
    denominator lands on psum partition 0 and y-rows on partitions
    64-127 in one M=128 matmul (LDW stays hidden).
  * Softmax epilogue: one DVE + one ACT copy move [96,1024] (y + denom
    rows) out of psum immediately, releasing the PV accumulator two steps
    into the next tile; reciprocal/broadcast/normalize then run entirely
    off the critical path on SBUF data.
  * Fine-grained software pipelining: projection blocks, epilogue chunks
    and output-projection halves are emitted as filler bundles between the
    ks-steps of the attention loop so the PE queue always holds
    independent work (keeps HAM at 2.4 GHz).
"""

from collections import deque

import numpy as np
import ml_dtypes

import concourse.bacc as bacc
import concourse.mybir as mybir
import concourse.tile as tile
from concourse.bass_utils import run_bass_kernel_spmd

P = 128
B, S, D = 2, 2048, 1024
H, DH = 16, 64
NROWS = B * S            # 4096 flattened rows
CH = 128                 # channels per core (2 heads)
RB = 512                 # row block for projections / q tiles
NRB = NROWS // RB        # 8
DSUB = D // P            # 8 contraction subtiles
KSUB = NROWS // P        # 32 k subtiles (128 rows each)
QT_PER_B = S // RB       # 4 q tiles per batch
ROPE_BASE = 10000.0

f32 = mybir.dt.float32
bf16 = mybir.dt.bfloat16
nbf16 = ml_dtypes.bfloat16

_CACHE = {}


def _build():
    nc = bacc.Bacc("TRN2", target_bir_lowering=False)

    xT_ext = nc.declare_dram_parameter("xT", [P, NRB * DSUB * RB], bf16,
                                       isOutput=False)
    wqT_ext = nc.declare_dram_parameter("wqT", [P, DSUB * CH], bf16,
                                        isOutput=False)
    wkT_ext = nc.declare_dram_parameter("wkT", [P, DSUB * CH], bf16,
                                        isOutput=False)
    wvT_ext = nc.declare_dram_parameter("wvT", [P, DSUB * CH], bf16,
                                        isOutput=False)
    woT_ext = nc.declare_dram_parameter("woT", [CH, D], bf16, isOutput=False)
    bq_ext = nc.declare_dram_parameter("bq", [CH, 1], f32, isOutput=False)
    bk_ext = nc.declare_dram_parameter("bk", [CH, 1], f32, isOutput=False)
    bvb_ext = nc.declare_dram_parameter("bvb", [P, CH], f32, isOutput=False)
    cc_ext = nc.declare_dram_parameter("cc2", [P, QT_PER_B * 2 * RB], bf16,
                                       isOutput=False)
    ss_ext = nc.declare_dram_parameter("ss2", [P, QT_PER_B * 2 * RB], bf16,
                                       isOutput=False)
    tri_ext = nc.declare_dram_parameter("tri", [P, 2 * P], bf16,
                                        isOutput=False)
    psw_ext = nc.declare_dram_parameter("pswm", [P, P], bf16, isOutput=False)
    out_ext = nc.declare_dram_parameter("out", [NROWS, D], bf16, isOutput=True)

    with tile.TileContext(nc) as tc:
        with (
            tc.tile_pool(name="const", bufs=1) as cpool,
            tc.tile_pool(name="xpool", bufs=NRB) as xpool,
            tc.tile_pool(name="big", bufs=1) as big,
            tc.tile_pool(name="work", bufs=3) as work,
            tc.tile_pool(name="ptp", bufs=8) as ptp,
            tc.tile_pool(name="epi", bufs=2) as epip,
            tc.tile_pool(name="obp", bufs=4) as obp,
            tc.tile_pool(name="stp", bufs=2, space="PSUM") as stp,
            tc.tile_pool(name="pvp", bufs=1, space="PSUM") as pvp,
            tc.tile_pool(name="smp", bufs=2, space="PSUM") as smp,
        ):
            # ---- input DMAs, ordered so block 0's deps land first ----
            wq_sb = cpool.tile([P, DSUB, CH], bf16, tag="wq")
            nc.sync.dma_start(wq_sb[:, 0:2].rearrange("p d c -> p (d c)"),
                              wqT_ext[:, 0:2 * CH])
            nc.sync.dma_start(wq_sb[:, 2:4].rearrange("p d c -> p (d c)"),
                              wqT_ext[:, 2 * CH:4 * CH])
            xTb = []

            def load_xt(rt, split=1):
                xt = xpool.tile([P, DSUB, RB], bf16, tag="xT", name=f"xT{rt}")
                o = rt * DSUB * RB
                w = DSUB // split
                for j in range(split):
                    nc.sync.dma_start(
                        xt[:, j * w:(j + 1) * w]
                        .rearrange("p d c -> p (d c)"),
                        xT_ext[:, o + j * w * RB:o + (j + 1) * w * RB])
                xTb.append(xt)

            load_xt(0, split=8)
            nc.sync.dma_start(wq_sb[:, 4:8].rearrange("p d c -> p (d c)"),
                              wqT_ext[:, 4 * CH:8 * CH])
            wk_sb = cpool.tile([P, DSUB, CH], bf16, tag="wk")
            nc.sync.dma_start(wk_sb[:].rearrange("p d c -> p (d c)"),
                              wkT_ext[:])
            bq_sb = cpool.tile([CH, 1], f32, tag="bq")
            nc.sync.dma_start(bq_sb[:], bq_ext[:])
            psw_sb = cpool.tile([P, P], bf16, tag="pswm")
            nc.sync.dma_start(psw_sb[:], psw_ext[:])
            bk_sb = cpool.tile([CH, 1], f32, tag="bk")
            nc.sync.dma_start(bk_sb[:], bk_ext[:])
            wv_sb = cpool.tile([P, DSUB, CH], bf16, tag="wv")
            nc.sync.dma_start(wv_sb[:].rearrange("p d c -> p (d c)"),
                              wvT_ext[:])
            bvb_sb = cpool.tile([P, CH], f32, tag="bvb")
            nc.sync.dma_start(bvb_sb[:], bvb_ext[:])
            cc_sb = cpool.tile([P, QT_PER_B, 2, RB], bf16, tag="cc")
            ss_sb = cpool.tile([P, QT_PER_B, 2, RB], bf16, tag="ss")
            for j in range(2):
                csl = slice(j * 4 * RB, (j + 1) * 4 * RB)
                nc.sync.dma_start(
                    cc_sb[:, 2 * j:2 * j + 2]
                    .rearrange("p a b c -> p (a b c)"), cc_ext[:, csl])
                nc.sync.dma_start(
                    ss_sb[:, 2 * j:2 * j + 2]
                    .rearrange("p a b c -> p (a b c)"), ss_ext[:, csl])
            tri_sb = cpool.tile([P, 2 * P], bf16, tag="tri")
            nc.sync.dma_start(tri_sb[:], tri_ext[:])
            load_xt(1, split=2)
            for rt in range(2, NRB):
                load_xt(rt)
            wo_sb = cpool.tile([CH, D], bf16, tag="wo")
            nc.sync.dma_start(wo_sb[:, 0:512], woT_ext[:, 0:512])
            nc.sync.dma_start(wo_sb[:, 512:1024], woT_ext[:, 512:1024])

            # ---- constants ----
            ones_f = cpool.tile([P, 2 * KSUB], f32, tag="onesf")
            nc.vector.memset(ones_f[:], 1.0)

            # ---- persistent activation tiles ----
            qkT = big.tile([P, NRB, 2, RB], bf16, tag="qkT")
            yT = big.tile([P, NROWS], bf16, tag="yT")
            v_sb = big.tile([P, KSUB, 2, CH], bf16, tag="v")
            # per head: [ones | 63 pad | 64 v-dims] = 128 cols
            nc.vector.tensor_copy(
                v_sb[:, :, :, 0:1].rearrange("p a b c -> p (a b c)"),
                ones_f[:])
            nc.vector.memset(v_sb[:, :, :, 1:64], 0.0)

            # ---------- phase A (projections + RoPE) as filler chunks ------
            def a_chunks(rt):
                pos = rt % QT_PER_B
                xt = xTb[rt]
                st_ = {}

                def a1():
                    pq = smp.tile([P, RB], f32, tag="sm", name=f"pq{rt}")
                    st_["pq"] = pq
                    for d in range(4):
                        nc.tensor.matmul(pq[:], wq_sb[:, d], xt[:, d],
                                         start=(d == 0), stop=False)

                def a2():
                    pq = st_["pq"]
                    for d in range(4, 8):
                        nc.tensor.matmul(pq[:], wq_sb[:, d], xt[:, d],
                                         start=False, stop=(d == 7))
                    praw = work.tile([P, 2, RB], bf16, tag="praw")
                    st_["praw"] = praw
                    nc.vector.tensor_scalar_add(praw[:, 0], pq[:],
                                                bq_sb[:, 0:1])

                def a3():
                    pk = smp.tile([P, RB], f32, tag="sm", name=f"pk{rt}")
                    st_["pk"] = pk
                    for d in range(4):
                        nc.tensor.matmul(pk[:], wk_sb[:, d], xt[:, d],
                                         start=(d == 0), stop=False)

                def a4():
                    pk = st_["pk"]
                    for d in range(4, 8):
                        nc.tensor.matmul(pk[:], wk_sb[:, d], xt[:, d],
                                         start=False, stop=(d == 7))
                    nc.vector.tensor_scalar_add(st_["praw"][:, 1], pk[:],
                                                bk_sb[:, 0:1])

                def a5():
                    # RoPE pair swap via PE permutation matmul
                    praw = st_["praw"]
                    swq = smp.tile([P, RB], f32, tag="sm", name=f"swq{rt}")
                    swk = smp.tile([P, RB], f32, tag="sm", name=f"swk{rt}")
                    st_["swq"], st_["swk"] = swq, swk
                    nc.tensor.matmul(swq[:], psw_sb[:], praw[:, 0],
                                     start=True, stop=True)
                    nc.tensor.matmul(swk[:], psw_sb[:], praw[:, 1],
                                     start=True, stop=True)

                def a6():
                    praw = st_["praw"]
                    t1 = work.tile([P, 2, RB], bf16, tag="t1")
                    st_["t1"] = t1
                    nc.vector.tensor_mul(t1[:], praw[:], cc_sb[:, pos])
                    t2q = work.tile([P, RB], bf16, tag="t2q")
                    nc.vector.tensor_mul(t2q[:], st_["swq"][:],
                                         ss_sb[:, pos, 0])
                    nc.vector.tensor_add(qkT[:, rt, 0], t1[:, 0], t2q[:])

                def a7():
                    t2k = work.tile([P, RB], bf16, tag="t2k")
                    nc.vector.tensor_mul(t2k[:], st_["swk"][:],
                                         ss_sb[:, pos, 1])
                    nc.vector.tensor_add(qkT[:, rt, 1], st_["t1"][:, 1],
                                         t2k[:])

                def av(blk):
                    def f():
                        vps = smp.tile([P, CH], f32, tag="sm",
                                       name=f"vps{rt}_{blk}")
                        for d in range(DSUB):
                            nc.tensor.matmul(
                                vps[:], xt[:, d, blk * P:(blk + 1) * P],
                                wv_sb[:, d], start=(d == 0), stop=(d == 7))
                        nc.vector.tensor_add(
                            v_sb[:, rt * 4 + blk, :, 64:128],
                            vps[:].rearrange("p (h c) -> p h c", h=2),
                            bvb_sb[:].rearrange("p (h c) -> p h c", h=2))
                    return f

                return [a1, a2, a3, a4, a5, a6, a7,
                        av(0), av(1), av(2), av(3)]

            # ---------- softmax epilogue as filler chunks ----------
            def epi_e0(state):
                # evacuate psum (denom row 0, y rows 64:128) immediately
                # so the PV accumulator bank frees early
                b, qt, qcols, pvm = state
                yraw = epip.tile([P, 1024], f32, tag="yraw")
                nc.vector.tensor_copy(yraw[:, 0:512], pvm[:, 0:512])
                nc.scalar.copy(yraw[:, 512:1024], pvm[:, 512:1024])
                return yraw

            def epi_chunks(state, yraw):
                b, qt, qcols, pvm = state
                st_ = {"yraw": yraw}

                def e1():
                    dn = epip.tile([1, 1024], f32, tag="dn")
                    nc.vector.reciprocal_approx_fast(dn[:],
                                                     st_["yraw"][0:1, :])
                    st_["dn"] = dn

                def e2():
                    # broadcast 1/denom across partitions on the idle gpsimd
                    rep = epip.tile([P, 1024], f32, tag="rep")
                    nc.gpsimd.partition_broadcast(rep[:], st_["dn"][0:1, :])
                    st_["rep"] = rep

                def e3():
                    ynorm = epip.tile([P, 1024], bf16, tag="ynorm")
                    nc.vector.tensor_mul(ynorm[64:128, :],
                                         st_["yraw"][64:128, :],
                                         st_["rep"][64:128, :])
                    nc.sync.dma_start(yT[0:64, qcols], ynorm[64:128, 0:512])
                    nc.sync.dma_start(yT[64:128, qcols],
                                      ynorm[64:128, 512:1024])

                return [e1, e2, e3]

            # ---------- phase D (output projection) as filler chunks ------
            def d_chunk(rr, half):
                def d1():
                    oph = smp.tile([P, 512], f32, tag="sm",
                                   name=f"op{rr}_{half}")
                    nc.tensor.matmul(oph[:], yT[:, rr * P:(rr + 1) * P],
                                     wo_sb[:, half * 512:(half + 1) * 512],
                                     start=True, stop=True)
                    ob = obp.tile([P, 512], bf16, tag="ob")
                    if (rr + half) % 2 == 0:
                        nc.vector.tensor_copy(ob[:], oph[:])
                    else:
                        nc.scalar.copy(ob[:], oph[:])
                    nc.sync.dma_start(
                        out_ext[rr * P:(rr + 1) * P,
                                half * 512:(half + 1) * 512], ob[:])
                return d1

            # ---------- attention q-tile with fillers ----------
            def phase_c(b, qt, fillers):
                qcols = slice(b * S + qt * RB, b * S + (qt + 1) * RB)
                rtq = b * QT_PER_B + qt
                nks = qt * 4 + 4
                pvm = pvp.tile([P, 1024], f32, tag="acc",
                               name=f"pvm{b}_{qt}")
                pts = {}

                def j0_of(ks):
                    m = ks - qt * 4
                    return m * P if m >= 1 else 0

                def emit_pv(kk):
                    jj = j0_of(kk)
                    ptk = pts.pop(kk)
                    first, last = (kk == 0), (kk == nks - 1)
                    for h in range(2):
                        csl = slice(h * 512 + jj, (h + 1) * 512)
                        # v_aug = [ones | pad | v]: denominator lands on psum
                        # partition 0, y rows on 64:128, in ONE M=128 matmul
                        nc.tensor.matmul(
                            pvm[:, csl],
                            v_sb[:, b * (S // P) + kk, h],
                            ptk[:, h, jj:], start=first, stop=last)

                for ks in range(nks):
                    rtk = b * QT_PER_B + ks // 4
                    c0 = (ks % 4) * P
                    m = ks - qt * 4
                    j0 = j0_of(ks)
                    st = stp.tile([P, 1024], f32, tag="st",
                                  name=f"st{b}_{qt}_{ks}")
                    stv = st[:].rearrange("p (h c) -> p h c", h=2)
                    pt = ptp.tile([P, 2, RB], bf16, tag="pt")
                    pts[ks] = pt
                    for h in range(2):
                        hsl = slice(h * 64, (h + 1) * 64)
                        nc.tensor.matmul(
                            st[:, h * 512 + j0:(h + 1) * 512],
                            qkT[hsl, rtk, 1, c0:c0 + P],
                            qkT[hsl, rtq, 0, j0:],
                            start=True, stop=True)
                    nc.scalar.activation(pt[:, :, j0:], stv[:, :, j0:],
                                         mybir.ActivationFunctionType.Exp)
                    if m >= 0:
                        triv = tri_sb[:].rearrange("p (a c) -> p a c", a=2)
                        nc.vector.tensor_mul(pt[:, :, j0:j0 + P],
                                             pt[:, :, j0:j0 + P], triv)
                    for _ in range(2):
                        if fillers:
                            fillers.popleft()()
                    if ks >= 2:
                        emit_pv(ks - 2)
                for kk in (nks - 2, nks - 1):
                    emit_pv(kk)
                return (b, qt, qcols, pvm)

            # ---------- final-tile epilogue: half-pipelined with d ---------
            def epi_final(state, yraw):
                b, qt, qcols, pvm = state
                rr0 = (b * QT_PER_B + qt) * 4
                dn = epip.tile([1, 1024], f32, tag="dn")
                rep = epip.tile([P, 1024], f32, tag="rep")
                ynorm = epip.tile([P, 1024], bf16, tag="ynorm")
                q0 = qcols.start
                dpend = []
                for j in range(2):
                    sl0 = slice(j * 256, (j + 1) * 256)
                    sl1 = slice(512 + j * 256, 512 + (j + 1) * 256)
                    for sl in (sl0, sl1):
                        nc.vector.reciprocal_approx_fast(dn[:, sl],
                                                         yraw[0:1, sl])
                        nc.gpsimd.partition_broadcast(rep[:, sl],
                                                      dn[0:1, sl])
                        nc.vector.tensor_mul(ynorm[64:128, sl],
                                             yraw[64:128, sl],
                                             rep[64:128, sl])
                    nc.sync.dma_start(yT[0:64, q0 + j * 256:q0 + j * 256
                                         + 256], ynorm[64:128, sl0])
                    nc.sync.dma_start(yT[64:128, q0 + j * 256:q0 + j * 256
                                          + 256], ynorm[64:128, sl1])
                    for _ in range(2):
                        if dq:
                            dq.popleft()()
                    for dd in dpend:
                        dd()
                    dpend = [d_chunk(rr0 + 2 * j + t, hh)
                             for t in range(2) for hh in range(2)]
                for dd in dpend:
                    dd()
                while dq:
                    dq.popleft()()

            # ---------- master schedule ----------
            dq = deque()          # deferred output-projection half chunks
            for ch in a_chunks(0):
                ch()
            for ch in a_chunks(1):
                ch()
            prev = None
            prev_yraw = None
            tile_order = [(0, 0), (0, 1), (0, 2), (0, 3),
                          (1, 1), (1, 2), (1, 3), (1, 0)]
            for rt, (b, qt) in enumerate(tile_order):
                fillers = deque()
                if prev is not None:
                    fillers.extend(epi_chunks(prev, prev_yraw))
                    rrp = (prev[0] * QT_PER_B + prev[1]) * 4
                    dq.extend(d_chunk(rrp + t, hh)
                              for t in range(4) for hh in range(2))
                if rt < NRB - 2:
                    fillers.extend(a_chunks(rt + 2))
                nks = qt * 4 + 4
                cap = 2 * nks + 4 if rt < NRB - 1 else nks + 4
                while len(fillers) < cap and dq:
                    fillers.append(dq.popleft())
                prev = phase_c(b, qt, fillers)
                prev_yraw = epi_e0(prev)
                while fillers:
                    fillers.popleft()()
            epi_final(prev, prev_yraw)

    nc.finalize()
    return nc


def _host_inputs():
    t = np.arange(32, dtype=np.float64)
    inv_freq = 1.0 / (ROPE_BASE ** (2.0 * t / DH))
    pos = np.arange(S, dtype=np.float64)
    ang = pos[None, :] * inv_freq[:, None]          # [32, S]
    cos32 = np.cos(ang).astype(np.float32)
    sin32 = np.sin(ang).astype(np.float32)
    cc = np.tile(cos32, (4, 1))                     # [128, S]
    ss = np.concatenate([-sin32, sin32, -sin32, sin32], axis=0)  # [128, S]
    # layout [P, pos_tile(4), dup(2), 512] so per-rt slices are contiguous
    cc2 = np.repeat(cc.reshape(P, 4, 1, RB), 2, axis=2).reshape(P, -1)
    ss2 = np.repeat(ss.reshape(P, 4, 1, RB), 2, axis=2).reshape(P, -1)

    ii = np.arange(P)[:, None]
    uu = np.arange(P)[None, :]
    tri = (uu >= ii).astype(np.float32)             # [128, 128]
    tri2 = np.concatenate([tri, tri], axis=1)       # [128, 256]

    perm64 = np.concatenate([np.arange(0, 64, 2), np.arange(1, 64, 2)])
    return cc2, ss2, tri2, perm64


def _in_maps(x, Wq, bq, Wk, bk, Wv, bv, Wo):
    cc2, ss2, tri2, perm64 = _host_inputs()
    pswm = np.zeros((P, P), dtype=np.float32)
    for m_ in range(P):
        k_ = (m_ & ~63) | ((m_ + 32) & 63)
        pswm[k_, m_] = 1.0
    pswm = pswm.astype(nbf16)
    x2 = np.ascontiguousarray(x.reshape(NROWS, D))
    # xT block-major: xT[p, rt, d, c] = x[512*rt + c, 128*d + p]
    xT = np.ascontiguousarray(
        x2.reshape(NRB, RB, DSUB, P).transpose(3, 0, 2, 1)
        .reshape(P, NRB * DSUB * RB)).astype(nbf16)
    perm128 = np.concatenate([perm64, perm64 + 64])
    cc2b = cc2.astype(nbf16)
    ss2b = ss2.astype(nbf16)
    tri2b = tri2.astype(nbf16)
    def warr(wT):
        # [D, CH] -> [P, DSUB*CH]: w[p, d*CH+c] = wT[d*P+p, c]
        return np.ascontiguousarray(
            wT.reshape(DSUB, P, CH).transpose(1, 0, 2)
            .reshape(P, DSUB * CH)).astype(nbf16)

    maps = []
    for c in range(8):
        sl = slice(c * CH, (c + 1) * CH)
        maps.append({
            "xT": xT,
            "wqT": warr((Wq[sl][perm128] * 0.125).T),
            "wkT": warr(Wk[sl][perm128].T),
            "wvT": warr(Wv[sl].T),
            "woT": np.ascontiguousarray(Wo[:, sl].T).astype(nbf16),
            "bq": (bq[sl][perm128] * 0.125).reshape(CH, 1).copy(),
            "bk": bk[sl][perm128].reshape(CH, 1).copy(),
            "bvb": np.ascontiguousarray(
                np.tile(bv[sl].reshape(1, CH), (P, 1))),
            "cc2": cc2b, "ss2": ss2b, "tri": tri2b, "pswm": pswm,
        })
    return maps


def kernel(x, Wq, bq, Wk, bk, Wv, bv, Wo, bo):
    x = np.asarray(x, dtype=np.float32)
    Wq = np.asarray(Wq, dtype=np.float32)
    Wk = np.asarray(Wk, dtype=np.float32)
    Wv = np.asarray(Wv, dtype=np.float32)
    Wo = np.asarray(Wo, dtype=np.float32)
    bq = np.asarray(bq, dtype=np.float32)
    bk = np.asarray(bk, dtype=np.float32)
    bv = np.asarray(bv, dtype=np.float32)
    bo = np.asarray(bo, dtype=np.float32)

    if "nc" not in _CACHE:
        _CACHE["nc"] = _build()
    nc = _CACHE["nc"]

    res = run_bass_kernel_spmd(nc, _in_maps(x, Wq, bq, Wk, bk, Wv, bv, Wo),
                               core_ids=list(range(8)))
    out = np.zeros((NROWS, D), dtype=np.float32)
    for r in res.results:
        out += r["out"].astype(np.float32)
    out += bo[None, :]
    return out.reshape(B, S, D)


# revision 17
# speedup vs baseline: 1.0080x; 1.0080x over previous
"""Multi-head attention (B=2, S=2048, D=1024, H=16, causal, interleaved RoPE)
on 8 Trainium2 NeuronCores.

Sharding: tensor-parallel over heads - 2 heads (128 channels) per core.
Each core computes its Q/K/V projections, RoPE, causal attention, and a
row-parallel partial of the output projection; the host sums the bf16
partials in fp32.

All matmuls in bf16 with fp32 PSUM accumulation. Key structure:
  * x^T is pre-transposed and cast to bf16 on the host (block-major layout)
    so no on-device transposes are needed for the q/k projections.
  * Q/K projection weights are host-permuted so each head's dims are
    [evens(32), odds(32)]; the RoPE pair-swap is then a 32-partition-block
    permutation done with ONE PE matmul against a 0/1 permutation matrix.
  * V is projected directly in transposed layout (lhsT = x^T block, rhs =
    Wv^T) as N=128 matmuls, so v_sb[k-pos, ch] needs no PE transposes and
    the psum->sbuf evacuation is a single contiguous copy fused with the
    bias add.
  * Attention uses the S^T layout: scores psum [k(128part), q(512)] via
    matmul(lhsT=K^T, rhs=Q^T); the two heads run CONCURRENTLY on disjoint
    PE row groups (K=64 each, auto tile_position from base partition),
    writing the two banks of one [128,1024] psum tile; ONE exp over both
    heads; causal masking multiplies only the [128,2x128] diagonal strip
    by a triangular constant (DVE). PV via matmul(lhsT=V_aug, rhs=P^T)
    with V_aug = [ones | 63 zero-pad | v dims] per head: the softmax
    denominator lands on psum partition 0 and y-rows on partitions
    64-127 in one M=128 matmul (LDW stays hidden).
  * Softmax epilogue: one DVE + one ACT copy move [96,1024] (y + denom
    rows) out of psum immediately, releasing the PV accumulator two steps
    into the next tile; reciprocal/broadcast/normalize then run entirely
    off the critical path on SBUF data.
  * Fine-grained software pipelining: projection blocks, epilogue chunks
    and output-projection halves are emitted as filler bundles between the
    ks-steps of the attention loop so the PE queue always holds
    independent work (keeps HAM at 2.4 GHz).
"""

from collections import deque

import numpy as np
import ml_dtypes

import concourse.bacc as bacc
import concourse.mybir as mybir
import concourse.tile as tile
from concourse.bass_utils import run_bass_kernel_spmd

P = 128
B, S, D = 2, 2048, 1024
H, DH = 16, 64
NROWS = B * S            # 4096 flattened rows
CH = 128                 # channels per core (2 heads)
RB = 512                 # row block for projections / q tiles
NRB = NROWS // RB        # 8
DSUB = D // P            # 8 contraction subtiles
KSUB = NROWS // P        # 32 k subtiles (128 rows each)
QT_PER_B = S // RB       # 4 q tiles per batch
ROPE_BASE = 10000.0

f32 = mybir.dt.float32
bf16 = mybir.dt.bfloat16
nbf16 = ml_dtypes.bfloat16

_CACHE = {}


def _build():
    nc = bacc.Bacc("TRN2", target_bir_lowering=False)

    xT_ext = nc.declare_dram_parameter("xT", [P, NRB * DSUB * RB], bf16,
                                       isOutput=False)
    wqT_ext = nc.declare_dram_parameter("wqT", [P, DSUB * CH], bf16,
                                        isOutput=False)
    wkT_ext = nc.declare_dram_parameter("wkT", [P, DSUB * CH], bf16,
                                        isOutput=False)
    wvT_ext = nc.declare_dram_parameter("wvT", [P, DSUB * CH], bf16,
                                        isOutput=False)
    woT_ext = nc.declare_dram_parameter("woT", [CH, D], bf16, isOutput=False)
    bq_ext = nc.declare_dram_parameter("bq", [CH, 1], f32, isOutput=False)
    bk_ext = nc.declare_dram_parameter("bk", [CH, 1], f32, isOutput=False)
    bvb_ext = nc.declare_dram_parameter("bvb", [P, CH], f32, isOutput=False)
    cc_ext = nc.declare_dram_parameter("cc2", [P, QT_PER_B * 2 * RB], bf16,
                                       isOutput=False)
    ss_ext = nc.declare_dram_parameter("ss2", [P, QT_PER_B * 2 * RB], bf16,
                                       isOutput=False)
    tri_ext = nc.declare_dram_parameter("tri", [P, 2 * P], bf16,
                                        isOutput=False)
    psw_ext = nc.declare_dram_parameter("pswm", [P, P], bf16, isOutput=False)
    out_ext = nc.declare_dram_parameter("out", [NROWS, D], bf16, isOutput=True)

    with tile.TileContext(nc) as tc:
        with (
            tc.tile_pool(name="const", bufs=1) as cpool,
            tc.tile_pool(name="xpool", bufs=NRB) as xpool,
            tc.tile_pool(name="big", bufs=1) as big,
            tc.tile_pool(name="work", bufs=3) as work,
            tc.tile_pool(name="ptp", bufs=8) as ptp,
            tc.tile_pool(name="epi", bufs=2) as epip,
            tc.tile_pool(name="obp", bufs=4) as obp,
            tc.tile_pool(name="stp", bufs=2, space="PSUM") as stp,
            tc.tile_pool(name="pvp", bufs=1, space="PSUM") as pvp,
            tc.tile_pool(name="smp", bufs=2, space="PSUM") as smp,
        ):
            # ---- input DMAs, ordered so block 0's deps land first ----
            wq_sb = cpool.tile([P, DSUB, CH], bf16, tag="wq")
            nc.sync.dma_start(wq_sb[:, 0:2].rearrange("p d c -> p (d c)"),
                              wqT_ext[:, 0:2 * CH])
            nc.sync.dma_start(wq_sb[:, 2:4].rearrange("p d c -> p (d c)"),
                              wqT_ext[:, 2 * CH:4 * CH])
            xTb = []

            def load_xt(rt, split=1):
                xt = xpool.tile([P, DSUB, RB], bf16, tag="xT", name=f"xT{rt}")
                o = rt * DSUB * RB
                w = DSUB // split
                for j in range(split):
                    nc.sync.dma_start(
                        xt[:, j * w:(j + 1) * w]
                        .rearrange("p d c -> p (d c)"),
                        xT_ext[:, o + j * w * RB:o + (j + 1) * w * RB])
                xTb.append(xt)

            load_xt(0, split=8)
            nc.sync.dma_start(wq_sb[:, 4:8].rearrange("p d c -> p (d c)"),
                              wqT_ext[:, 4 * CH:8 * CH])
            wk_sb = cpool.tile([P, DSUB, CH], bf16, tag="wk")
            nc.sync.dma_start(wk_sb[:].rearrange("p d c -> p (d c)"),
                              wkT_ext[:])
            bq_sb = cpool.tile([CH, 1], f32, tag="bq")
            nc.sync.dma_start(bq_sb[:], bq_ext[:])
            psw_sb = cpool.tile([P, P], bf16, tag="pswm")
            nc.sync.dma_start(psw_sb[:], psw_ext[:])
            bk_sb = cpool.tile([CH, 1], f32, tag="bk")
            nc.sync.dma_start(bk_sb[:], bk_ext[:])
            wv_sb = cpool.tile([P, DSUB, CH], bf16, tag="wv")
            nc.sync.dma_start(wv_sb[:].rearrange("p d c -> p (d c)"),
                              wvT_ext[:])
            bvb_sb = cpool.tile([P, CH], f32, tag="bvb")
            nc.sync.dma_start(bvb_sb[:], bvb_ext[:])
            cc_sb = cpool.tile([P, QT_PER_B, 2, RB], bf16, tag="cc")
            ss_sb = cpool.tile([P, QT_PER_B, 2, RB], bf16, tag="ss")
            for j in range(2):
                csl = slice(j * 4 * RB, (j + 1) * 4 * RB)
                nc.sync.dma_start(
                    cc_sb[:, 2 * j:2 * j + 2]
                    .rearrange("p a b c -> p (a b c)"), cc_ext[:, csl])
                nc.sync.dma_start(
                    ss_sb[:, 2 * j:2 * j + 2]
                    .rearrange("p a b c -> p (a b c)"), ss_ext[:, csl])
            tri_sb = cpool.tile([P, 2 * P], bf16, tag="tri")
            nc.sync.dma_start(tri_sb[:], tri_ext[:])
            load_xt(1, split=2)
            for rt in range(2, NRB):
                load_xt(rt)
            wo_sb = cpool.tile([CH, D], bf16, tag="wo")
            nc.sync.dma_start(wo_sb[:, 0:512], woT_ext[:, 0:512])
            nc.sync.dma_start(wo_sb[:, 512:1024], woT_ext[:, 512:1024])

            # ---- constants ----
            ones_f = cpool.tile([P, 2 * KSUB], f32, tag="onesf")
            nc.vector.memset(ones_f[:], 1.0)

            # ---- persistent activation tiles ----
            qkT = big.tile([P, NRB, 2, RB], bf16, tag="qkT")
            yT = big.tile([P, NROWS], bf16, tag="yT")
            v_sb = big.tile([P, KSUB, 2, CH], bf16, tag="v")
            # per head: [ones | 63 pad | 64 v-dims] = 128 cols
            nc.vector.tensor_copy(
                v_sb[:, :, :, 0:1].rearrange("p a b c -> p (a b c)"),
                ones_f[:])
            nc.vector.memset(v_sb[:, :, :, 1:64], 0.0)

            # ---------- phase A (projections + RoPE) as filler chunks ------
            def a_chunks(rt):
                pos = rt % QT_PER_B
                xt = xTb[rt]
                st_ = {}

                def a1():
                    pq = smp.tile([P, RB], f32, tag="sm", name=f"pq{rt}")
                    st_["pq"] = pq
                    for d in range(4):
                        nc.tensor.matmul(pq[:], wq_sb[:, d], xt[:, d],
                                         start=(d == 0), stop=False)

                def a2():
                    pq = st_["pq"]
                    for d in range(4, 8):
                        nc.tensor.matmul(pq[:], wq_sb[:, d], xt[:, d],
                                         start=False, stop=(d == 7))
                    praw = work.tile([P, 2, RB], bf16, tag="praw")
                    st_["praw"] = praw
                    nc.vector.tensor_scalar_add(praw[:, 0], pq[:],
                                                bq_sb[:, 0:1])

                def a3():
                    pk = smp.tile([P, RB], f32, tag="sm", name=f"pk{rt}")
                    st_["pk"] = pk
                    for d in range(4):
                        nc.tensor.matmul(pk[:], wk_sb[:, d], xt[:, d],
                                         start=(d == 0), stop=False)

                def a4():
                    pk = st_["pk"]
                    for d in range(4, 8):
                        nc.tensor.matmul(pk[:], wk_sb[:, d], xt[:, d],
                                         start=False, stop=(d == 7))
                    nc.vector.tensor_scalar_add(st_["praw"][:, 1], pk[:],
                                                bk_sb[:, 0:1])

                def a5():
                    # RoPE pair swap via PE permutation matmul
                    praw = st_["praw"]
                    swq = smp.tile([P, RB], f32, tag="sm", name=f"swq{rt}")
                    swk = smp.tile([P, RB], f32, tag="sm", name=f"swk{rt}")
                    st_["swq"], st_["swk"] = swq, swk
                    nc.tensor.matmul(swq[:], psw_sb[:], praw[:, 0],
                                     start=True, stop=True)
                    nc.tensor.matmul(swk[:], psw_sb[:], praw[:, 1],
                                     start=True, stop=True)

                def a6():
                    praw = st_["praw"]
                    t1 = work.tile([P, 2, RB], bf16, tag="t1")
                    st_["t1"] = t1
                    nc.vector.tensor_mul(t1[:], praw[:], cc_sb[:, pos])
                    t2q = work.tile([P, RB], bf16, tag="t2q")
                    nc.vector.tensor_mul(t2q[:], st_["swq"][:],
                                         ss_sb[:, pos, 0])
                    nc.vector.tensor_add(qkT[:, rt, 0], t1[:, 0], t2q[:])

                def a7():
                    t2k = work.tile([P, RB], bf16, tag="t2k")
                    nc.vector.tensor_mul(t2k[:], st_["swk"][:],
                                         ss_sb[:, pos, 1])
                    nc.vector.tensor_add(qkT[:, rt, 1], st_["t1"][:, 1],
                                         t2k[:])

                def av(blk):
                    def f():
                        vps = smp.tile([P, CH], f32, tag="sm",
                                       name=f"vps{rt}_{blk}")
                        for d in range(DSUB):
                            nc.tensor.matmul(
                                vps[:], xt[:, d, blk * P:(blk + 1) * P],
                                wv_sb[:, d], start=(d == 0), stop=(d == 7))
                        nc.vector.tensor_add(
                            v_sb[:, rt * 4 + blk, :, 64:128],
                            vps[:].rearrange("p (h c) -> p h c", h=2),
                            bvb_sb[:].rearrange("p (h c) -> p h c", h=2))
                    return f

                return [a1, a2, a3, a4, a5, a6, a7,
                        av(0), av(1), av(2), av(3)]

            # ---------- softmax epilogue as filler chunks ----------
            def epi_e0(state):
                # evacuate psum (denom row 0, y rows 64:128) immediately
                # so the PV accumulator bank frees early
                b, qt, qcols, pvm = state
                yraw = epip.tile([P, 1024], f32, tag="yraw")
                nc.vector.tensor_copy(yraw[:, 0:512], pvm[:, 0:512])
                nc.scalar.copy(yraw[:, 512:1024], pvm[:, 512:1024])
                return yraw

            def epi_chunks(state, yraw):
                b, qt, qcols, pvm = state
                st_ = {"yraw": yraw}

                def e1():
                    dn = epip.tile([1, 1024], f32, tag="dn")
                    nc.vector.reciprocal_approx_fast(dn[:],
                                                     st_["yraw"][0:1, :])
                    st_["dn"] = dn

                def e2():
                    # broadcast 1/denom across partitions on the idle gpsimd
                    rep = epip.tile([P, 1024], f32, tag="rep")
                    nc.gpsimd.partition_broadcast(rep[:], st_["dn"][0:1, :])
                    st_["rep"] = rep

                def e3():
                    ynorm = epip.tile([P, 1024], bf16, tag="ynorm")
                    nc.vector.tensor_mul(ynorm[64:128, :],
                                         st_["yraw"][64:128, :],
                                         st_["rep"][64:128, :])
                    nc.sync.dma_start(yT[0:64, qcols], ynorm[64:128, 0:512])
                    nc.sync.dma_start(yT[64:128, qcols],
                                      ynorm[64:128, 512:1024])

                return [e1, e2, e3]

            # ---------- phase D (output projection) as filler chunks ------
            def d_chunk(rr, half):
                def d1():
                    oph = smp.tile([P, 512], f32, tag="sm",
                                   name=f"op{rr}_{half}")
                    nc.tensor.matmul(oph[:], yT[:, rr * P:(rr + 1) * P],
                                     wo_sb[:, half * 512:(half + 1) * 512],
                                     start=True, stop=True)
                    ob = obp.tile([P, 512], bf16, tag="ob")
                    if (rr + half) % 2 == 0:
                        nc.vector.tensor_copy(ob[:], oph[:])
                    else:
                        nc.scalar.copy(ob[:], oph[:])
                    nc.sync.dma_start(
                        out_ext[rr * P:(rr + 1) * P,
                                half * 512:(half + 1) * 512], ob[:])
                return d1

            # ---------- attention q-tile with fillers ----------
            def phase_c(b, qt, fillers):
                qcols = slice(b * S + qt * RB, b * S + (qt + 1) * RB)
                rtq = b * QT_PER_B + qt
                nks = qt * 4 + 4
                pvm = pvp.tile([P, 1024], f32, tag="acc",
                               name=f"pvm{b}_{qt}")
                pts = {}

                def j0_of(ks):
                    m = ks - qt * 4
                    return m * P if m >= 1 else 0

                def emit_pv(kk):
                    jj = j0_of(kk)
                    ptk = pts.pop(kk)
                    first, last = (kk == 0), (kk == nks - 1)
                    for h in range(2):
                        csl = slice(h * 512 + jj, (h + 1) * 512)
                        # v_aug = [ones | pad | v]: denominator lands on psum
                        # partition 0, y rows on 64:128, in ONE M=128 matmul
                        nc.tensor.matmul(
                            pvm[:, csl],
                            v_sb[:, b * (S // P) + kk, h],
                            ptk[:, h, jj:], start=first, stop=last)

                for ks in range(nks):
                    rtk = b * QT_PER_B + ks // 4
                    c0 = (ks % 4) * P
                    m = ks - qt * 4
                    j0 = j0_of(ks)
                    st = stp.tile([P, 1024], f32, tag="st",
                                  name=f"st{b}_{qt}_{ks}")
                    stv = st[:].rearrange("p (h c) -> p h c", h=2)
                    pt = ptp.tile([P, 2, RB], bf16, tag="pt")
                    pts[ks] = pt
                    for h in range(2):
                        hsl = slice(h * 64, (h + 1) * 64)
                        nc.tensor.matmul(
                            st[:, h * 512 + j0:(h + 1) * 512],
                            qkT[hsl, rtk, 1, c0:c0 + P],
                            qkT[hsl, rtq, 0, j0:],
                            start=True, stop=True)
                    nc.scalar.activation(pt[:, :, j0:], stv[:, :, j0:],
                                         mybir.ActivationFunctionType.Exp)
                    if m >= 0:
                        triv = tri_sb[:].rearrange("p (a c) -> p a c", a=2)
                        nc.vector.tensor_mul(pt[:, :, j0:j0 + P],
                                             pt[:, :, j0:j0 + P], triv)
                    for _ in range(2):
                        if fillers:
                            fillers.popleft()()
                    if ks >= 2:
                        emit_pv(ks - 2)
                for kk in (nks - 2, nks - 1):
                    emit_pv(kk)
                return (b, qt, qcols, pvm)

            # ---------- final-tile epilogue: half-pipelined with d ---------
            def epi_final(state, yraw):
                b, qt, qcols, pvm = state
                rr0 = (b * QT_PER_B + qt) * 4
                dn = epip.tile([1, 1024], f32, tag="dn")
                rep = epip.tile([P, 1024], f32, tag="rep")
                ynorm = epip.tile([P, 1024], bf16, tag="ynorm")
                q0 = qcols.start
                dpend = []
                for j in range(2):
                    sl0 = slice(j * 256, (j + 1) * 256)
                    sl1 = slice(512 + j * 256, 512 + (j + 1) * 256)
                    for sl in (sl0, sl1):
                        nc.vector.reciprocal_approx_fast(dn[:, sl],
                                                         yraw[0:1, sl])
                        nc.gpsimd.partition_broadcast(rep[:, sl],
                                                      dn[0:1, sl])
                        nc.vector.tensor_mul(ynorm[64:128, sl],
                                             yraw[64:128, sl],
                                             rep[64:128, sl])
                    nc.sync.dma_start(yT[0:64, q0 + j * 256:q0 + j * 256
                                         + 256], ynorm[64:128, sl0])
                    nc.sync.dma_start(yT[64:128, q0 + j * 256:q0 + j * 256
                                          + 256], ynorm[64:128, sl1])
                    for _ in range(2):
                        if dq:
                            dq.popleft()()
                    for dd in dpend:
                        dd()
                    dpend = [d_chunk(rr0 + 2 * j + t, hh)
                             for t in range(2) for hh in range(2)]
                for dd in dpend:
                    dd()
                while dq:
                    dq.popleft()()

            # ---------- master schedule ----------
            dq = deque()          # deferred output-projection half chunks
            for ch in a_chunks(0):
                ch()
            for ch in a_chunks(1):
                ch()
            prev = None
            prev_yraw = None
            tile_order = [(0, 0), (0, 1), (0, 2), (0, 3),
                          (1, 0), (1, 1), (1, 2), (1, 3)]
            for rt, (b, qt) in enumerate(tile_order):
                fillers = deque()
                if prev is not None:
                    fillers.extend(epi_chunks(prev, prev_yraw))
                    rrp = (prev[0] * QT_PER_B + prev[1]) * 4
                    dq.extend(d_chunk(rrp + t, hh)
                              for t in range(4) for hh in range(2))
                if rt < NRB - 2:
                    fillers.extend(a_chunks(rt + 2))
                nks = qt * 4 + 4
                cap = 2 * nks + 4 if rt < NRB - 1 else nks + 4
                while len(fillers) < cap and dq:
                    fillers.append(dq.popleft())
                prev = phase_c(b, qt, fillers)
                prev_yraw = epi_e0(prev)
                while fillers:
                    fillers.popleft()()
            epi_final(prev, prev_yraw)

    nc.finalize()
    return nc


def _host_inputs():
    t = np.arange(32, dtype=np.float64)
    inv_freq = 1.0 / (ROPE_BASE ** (2.0 * t / DH))
    pos = np.arange(S, dtype=np.float64)
    ang = pos[None, :] * inv_freq[:, None]          # [32, S]
    cos32 = np.cos(ang).astype(np.float32)
    sin32 = np.sin(ang).astype(np.float32)
    cc = np.tile(cos32, (4, 1))                     # [128, S]
    ss = np.concatenate([-sin32, sin32, -sin32, sin32], axis=0)  # [128, S]
    # layout [P, pos_tile(4), dup(2), 512] so per-rt slices are contiguous
    cc2 = np.repeat(cc.reshape(P, 4, 1, RB), 2, axis=2).reshape(P, -1)
    ss2 = np.repeat(ss.reshape(P, 4, 1, RB), 2, axis=2).reshape(P, -1)

    ii = np.arange(P)[:, None]
    uu = np.arange(P)[None, :]
    tri = (uu >= ii).astype(np.float32)             # [128, 128]
    tri2 = np.concatenate([tri, tri], axis=1)       # [128, 256]

    perm64 = np.concatenate([np.arange(0, 64, 2), np.arange(1, 64, 2)])
    return cc2, ss2, tri2, perm64


def _in_maps(x, Wq, bq, Wk, bk, Wv, bv, Wo):
    cc2, ss2, tri2, perm64 = _host_inputs()
    pswm = np.zeros((P, P), dtype=np.float32)
    for m_ in range(P):
        k_ = (m_ & ~63) | ((m_ + 32) & 63)
        pswm[k_, m_] = 1.0
    pswm = pswm.astype(nbf16)
    x2 = np.ascontiguousarray(x.reshape(NROWS, D))
    # xT block-major: xT[p, rt, d, c] = x[512*rt + c, 128*d + p]
    xT = np.ascontiguousarray(
        x2.reshape(NRB, RB, DSUB, P).transpose(3, 0, 2, 1)
        .reshape(P, NRB * DSUB * RB)).astype(nbf16)
    perm128 = np.concatenate([perm64, perm64 + 64])
    cc2b = cc2.astype(nbf16)
    ss2b = ss2.astype(nbf16)
    tri2b = tri2.astype(nbf16)
    def warr(wT):
        # [D, CH] -> [P, DSUB*CH]: w[p, d*CH+c] = wT[d*P+p, c]
        return np.ascontiguousarray(
            wT.reshape(DSUB, P, CH).transpose(1, 0, 2)
            .reshape(P, DSUB * CH)).astype(nbf16)

    maps = []
    for c in range(8):
        sl = slice(c * CH, (c + 1) * CH)
        maps.append({
            "xT": xT,
            "wqT": warr((Wq[sl][perm128] * 0.125).T),
            "wkT": warr(Wk[sl][perm128].T),
            "wvT": warr(Wv[sl].T),
            "woT": np.ascontiguousarray(Wo[:, sl].T).astype(nbf16),
            "bq": (bq[sl][perm128] * 0.125).reshape(CH, 1).copy(),
            "bk": bk[sl][perm128].reshape(CH, 1).copy(),
            "bvb": np.ascontiguousarray(
                np.tile(bv[sl].reshape(1, CH), (P, 1))),
            "cc2": cc2b, "ss2": ss2b, "tri": tri2b, "pswm": pswm,
        })
    return maps


def kernel(x, Wq, bq, Wk, bk, Wv, bv, Wo, bo):
    x = np.asarray(x, dtype=np.float32)
    Wq = np.asarray(Wq, dtype=np.float32)
    Wk = np.asarray(Wk, dtype=np.float32)
    Wv = np.asarray(Wv, dtype=np.float32)
    Wo = np.asarray(Wo, dtype=np.float32)
    bq = np.asarray(bq, dtype=np.float32)
    bk = np.asarray(bk, dtype=np.float32)
    bv = np.asarray(bv, dtype=np.float32)
    bo = np.asarray(bo, dtype=np.float32)

    if "nc" not in _CACHE:
        _CACHE["nc"] = _build()
    nc = _CACHE["nc"]

    res = run_bass_kernel_spmd(nc, _in_maps(x, Wq, bq, Wk, bk, Wv, bv, Wo),
                               core_ids=list(range(8)))
    out = np.zeros((NROWS, D), dtype=np.float32)
    for r in res.results:
        out += r["out"].astype(np.float32)
    out += bo[None, :]
    return out.reshape(B, S, D)


# revision 18
# speedup vs baseline: 1.0256x; 1.0175x over previous
"""Multi-head attention (B=2, S=2048, D=1024, H=16, causal, interleaved RoPE)
on 8 Trainium2 NeuronCores.

Sharding: tensor-parallel over heads - 2 heads (128 channels) per core.
Each core computes its Q/K/V projections, RoPE, causal attention, and a
row-parallel partial of the output projection; the host sums the bf16
partials in fp32.

All matmuls in bf16 with fp32 PSUM accumulation. Key structure:
  * x^T is pre-transposed and cast to bf16 on the host (block-major layout)
    so no on-device transposes are needed for the q/k projections.
  * Q/K projection weights are host-permuted so each head's dims are
    [evens(32), odds(32)]; the RoPE pair-swap is then a 32-partition-block
    permutation done with ONE PE matmul against a 0/1 permutation matrix.
  * V is projected directly in transposed layout (lhsT = x^T block, rhs =
    Wv^T) as N=128 matmuls, so v_sb[k-pos, ch] needs no PE transposes and
    the psum->sbuf evacuation is a single contiguous copy fused with the
    bias add.
  * Attention uses the S^T layout: scores psum [k(128part), q(512)] via
    matmul(lhsT=K^T, rhs=Q^T); the two heads run CONCURRENTLY on disjoint
    PE row groups (K=64 each, auto tile_position from base partition),
    writing the two banks of one [128,1024] psum tile; ONE exp over both
    heads; causal masking multiplies only the [128,2x128] diagonal strip
    by a triangular constant (DVE). PV via matmul(lhsT=V_aug, rhs=P^T)
    with V_aug = [ones | 63 zero-pad | v dims] per head: the softmax
    denominator lands on psum partition 0 and y-rows on partitions
    64-127 in one M=128 matmul (LDW stays hidden).
  * Softmax epilogue: one DVE + one ACT copy move [96,1024] (y + denom
    rows) out of psum immediately, releasing the PV accumulator two steps
    into the next tile; reciprocal/broadcast/normalize then run entirely
    off the critical path on SBUF data.
  * Fine-grained software pipelining: projection blocks, epilogue chunks
    and output-projection halves are emitted as filler bundles between the
    ks-steps of the attention loop so the PE queue always holds
    independent work (keeps HAM at 2.4 GHz).
"""

from collections import deque

import numpy as np
import ml_dtypes

import concourse.bacc as bacc
import concourse.mybir as mybir
import concourse.tile as tile
from concourse.bass_utils import run_bass_kernel_spmd

P = 128
B, S, D = 2, 2048, 1024
H, DH = 16, 64
NROWS = B * S            # 4096 flattened rows
CH = 128                 # channels per core (2 heads)
RB = 512                 # row block for projections / q tiles
NRB = NROWS // RB        # 8
DSUB = D // P            # 8 contraction subtiles
KSUB = NROWS // P        # 32 k subtiles (128 rows each)
QT_PER_B = S // RB       # 4 q tiles per batch
ROPE_BASE = 10000.0

f32 = mybir.dt.float32
bf16 = mybir.dt.bfloat16
nbf16 = ml_dtypes.bfloat16

_CACHE = {}


def _build():
    nc = bacc.Bacc("TRN2", target_bir_lowering=False)

    xT_ext = nc.declare_dram_parameter("xT", [P, NRB * DSUB * RB], bf16,
                                       isOutput=False)
    wqT_ext = nc.declare_dram_parameter("wqT", [P, DSUB * CH], bf16,
                                        isOutput=False)
    wkT_ext = nc.declare_dram_parameter("wkT", [P, DSUB * CH], bf16,
                                        isOutput=False)
    wvT_ext = nc.declare_dram_parameter("wvT", [P, DSUB * CH], bf16,
                                        isOutput=False)
    woT_ext = nc.declare_dram_parameter("woT", [CH, D], bf16, isOutput=False)
    bq_ext = nc.declare_dram_parameter("bq", [CH, 1], f32, isOutput=False)
    bk_ext = nc.declare_dram_parameter("bk", [CH, 1], f32, isOutput=False)
    bvb_ext = nc.declare_dram_parameter("bvb", [P, CH], f32, isOutput=False)
    cc_ext = nc.declare_dram_parameter("cc2", [P, QT_PER_B * 2 * RB], bf16,
                                       isOutput=False)
    ss_ext = nc.declare_dram_parameter("ss2", [P, QT_PER_B * 2 * RB], bf16,
                                       isOutput=False)
    tri_ext = nc.declare_dram_parameter("tri", [P, 2 * P], bf16,
                                        isOutput=False)
    psw_ext = nc.declare_dram_parameter("pswm", [P, P], bf16, isOutput=False)
    out_ext = nc.declare_dram_parameter("out", [NROWS, D], bf16, isOutput=True)

    with tile.TileContext(nc) as tc:
        with (
            tc.tile_pool(name="const", bufs=1) as cpool,
            tc.tile_pool(name="xpool", bufs=NRB) as xpool,
            tc.tile_pool(name="big", bufs=1) as big,
            tc.tile_pool(name="work", bufs=3) as work,
            tc.tile_pool(name="ptp", bufs=8) as ptp,
            tc.tile_pool(name="epi", bufs=2) as epip,
            tc.tile_pool(name="obp", bufs=4) as obp,
            tc.tile_pool(name="stp", bufs=2, space="PSUM") as stp,
            tc.tile_pool(name="pvp", bufs=1, space="PSUM") as pvp,
            tc.tile_pool(name="smp", bufs=2, space="PSUM") as smp,
        ):
            # ---- HAM warmup: dependency-free dummy matmuls run during the
            # input-DMA ramp so real compute starts at the 2.4 GHz clock ----
            wf = cpool.tile([P, P], f32, tag="warmf")
            nc.vector.memset(wf[:], 0.0)
            warm = cpool.tile([P, P], bf16, tag="warm")
            nc.vector.tensor_copy(warm[:], wf[:])
            wps = smp.tile([P, 512], f32, tag="sm", name="warmps")
            for _ in range(40):
                nc.tensor.matmul(wps[:, 0:P], warm[:], warm[:],
                                 start=True, stop=True)

            # ---- input DMAs, ordered so block 0's deps land first ----
            wq_sb = cpool.tile([P, DSUB, CH], bf16, tag="wq")
            nc.sync.dma_start(wq_sb[:, 0:2].rearrange("p d c -> p (d c)"),
                              wqT_ext[:, 0:2 * CH])
            nc.sync.dma_start(wq_sb[:, 2:4].rearrange("p d c -> p (d c)"),
                              wqT_ext[:, 2 * CH:4 * CH])
            xTb = []

            def load_xt(rt, split=1):
                xt = xpool.tile([P, DSUB, RB], bf16, tag="xT", name=f"xT{rt}")
                o = rt * DSUB * RB
                w = DSUB // split
                for j in range(split):
                    nc.sync.dma_start(
                        xt[:, j * w:(j + 1) * w]
                        .rearrange("p d c -> p (d c)"),
                        xT_ext[:, o + j * w * RB:o + (j + 1) * w * RB])
                xTb.append(xt)

            load_xt(0, split=8)
            nc.sync.dma_start(wq_sb[:, 4:8].rearrange("p d c -> p (d c)"),
                              wqT_ext[:, 4 * CH:8 * CH])
            wk_sb = cpool.tile([P, DSUB, CH], bf16, tag="wk")
            nc.sync.dma_start(wk_sb[:].rearrange("p d c -> p (d c)"),
                              wkT_ext[:])
            bq_sb = cpool.tile([CH, 1], f32, tag="bq")
            nc.sync.dma_start(bq_sb[:], bq_ext[:])
            psw_sb = cpool.tile([P, P], bf16, tag="pswm")
            nc.sync.dma_start(psw_sb[:], psw_ext[:])
            bk_sb = cpool.tile([CH, 1], f32, tag="bk")
            nc.sync.dma_start(bk_sb[:], bk_ext[:])
            wv_sb = cpool.tile([P, DSUB, CH], bf16, tag="wv")
            nc.sync.dma_start(wv_sb[:].rearrange("p d c -> p (d c)"),
                              wvT_ext[:])
            bvb_sb = cpool.tile([P, CH], f32, tag="bvb")
            nc.sync.dma_start(bvb_sb[:], bvb_ext[:])
            cc_sb = cpool.tile([P, QT_PER_B, 2, RB], bf16, tag="cc")
            ss_sb = cpool.tile([P, QT_PER_B, 2, RB], bf16, tag="ss")
            for j in range(2):
                csl = slice(j * 4 * RB, (j + 1) * 4 * RB)
                nc.sync.dma_start(
                    cc_sb[:, 2 * j:2 * j + 2]
                    .rearrange("p a b c -> p (a b c)"), cc_ext[:, csl])
                nc.sync.dma_start(
                    ss_sb[:, 2 * j:2 * j + 2]
                    .rearrange("p a b c -> p (a b c)"), ss_ext[:, csl])
            tri_sb = cpool.tile([P, 2 * P], bf16, tag="tri")
            nc.sync.dma_start(tri_sb[:], tri_ext[:])
            load_xt(1, split=2)
            for rt in range(2, NRB):
                load_xt(rt)
            wo_sb = cpool.tile([CH, D], bf16, tag="wo")
            nc.sync.dma_start(wo_sb[:, 0:512], woT_ext[:, 0:512])
            nc.sync.dma_start(wo_sb[:, 512:1024], woT_ext[:, 512:1024])

            # ---- constants ----
            ones_f = cpool.tile([P, 2 * KSUB], f32, tag="onesf")
            nc.vector.memset(ones_f[:], 1.0)

            # ---- persistent activation tiles ----
            qkT = big.tile([P, NRB, 2, RB], bf16, tag="qkT")
            yT = big.tile([P, NROWS], bf16, tag="yT")
            v_sb = big.tile([P, KSUB, 2, CH], bf16, tag="v")
            # per head: [ones | 63 pad | 64 v-dims] = 128 cols
            nc.vector.tensor_copy(
                v_sb[:, :, :, 0:1].rearrange("p a b c -> p (a b c)"),
                ones_f[:])
            nc.vector.memset(v_sb[:, :, :, 1:64], 0.0)

            # ---------- phase A (projections + RoPE) as filler chunks ------
            def a_chunks(rt):
                pos = rt % QT_PER_B
                xt = xTb[rt]
                st_ = {}

                def a1():
                    pq = smp.tile([P, RB], f32, tag="sm", name=f"pq{rt}")
                    st_["pq"] = pq
                    for d in range(4):
                        nc.tensor.matmul(pq[:], wq_sb[:, d], xt[:, d],
                                         start=(d == 0), stop=False)

                def a2():
                    pq = st_["pq"]
                    for d in range(4, 8):
                        nc.tensor.matmul(pq[:], wq_sb[:, d], xt[:, d],
                                         start=False, stop=(d == 7))
                    praw = work.tile([P, 2, RB], bf16, tag="praw")
                    st_["praw"] = praw
                    nc.vector.tensor_scalar_add(praw[:, 0], pq[:],
                                                bq_sb[:, 0:1])

                def a3():
                    pk = smp.tile([P, RB], f32, tag="sm", name=f"pk{rt}")
                    st_["pk"] = pk
                    for d in range(4):
                        nc.tensor.matmul(pk[:], wk_sb[:, d], xt[:, d],
                                         start=(d == 0), stop=False)

                def a4():
                    pk = st_["pk"]
                    for d in range(4, 8):
                        nc.tensor.matmul(pk[:], wk_sb[:, d], xt[:, d],
                                         start=False, stop=(d == 7))
                    nc.vector.tensor_scalar_add(st_["praw"][:, 1], pk[:],
                                                bk_sb[:, 0:1])

                def a5():
                    # RoPE pair swap via PE permutation matmul
                    praw = st_["praw"]
                    swq = smp.tile([P, RB], f32, tag="sm", name=f"swq{rt}")
                    swk = smp.tile([P, RB], f32, tag="sm", name=f"swk{rt}")
                    st_["swq"], st_["swk"] = swq, swk
                    nc.tensor.matmul(swq[:], psw_sb[:], praw[:, 0],
                                     start=True, stop=True)
                    nc.tensor.matmul(swk[:], psw_sb[:], praw[:, 1],
                                     start=True, stop=True)

                def a6():
                    praw = st_["praw"]
                    t1 = work.tile([P, 2, RB], bf16, tag="t1")
                    st_["t1"] = t1
                    nc.vector.tensor_mul(t1[:], praw[:], cc_sb[:, pos])
                    t2q = work.tile([P, RB], bf16, tag="t2q")
                    nc.vector.tensor_mul(t2q[:], st_["swq"][:],
                                         ss_sb[:, pos, 0])
                    nc.vector.tensor_add(qkT[:, rt, 0], t1[:, 0], t2q[:])

                def a7():
                    t2k = work.tile([P, RB], bf16, tag="t2k")
                    nc.vector.tensor_mul(t2k[:], st_["swk"][:],
                                         ss_sb[:, pos, 1])
                    nc.vector.tensor_add(qkT[:, rt, 1], st_["t1"][:, 1],
                                         t2k[:])

                def av(blk):
                    def f():
                        vps = smp.tile([P, CH], f32, tag="sm",
                                       name=f"vps{rt}_{blk}")
                        for d in range(DSUB):
                            nc.tensor.matmul(
                                vps[:], xt[:, d, blk * P:(blk + 1) * P],
                                wv_sb[:, d], start=(d == 0), stop=(d == 7))
                        nc.vector.tensor_add(
                            v_sb[:, rt * 4 + blk, :, 64:128],
                            vps[:].rearrange("p (h c) -> p h c", h=2),
                            bvb_sb[:].rearrange("p (h c) -> p h c", h=2))
                    return f

                return [a1, a2, a3, a4, a5, a6, a7,
                        av(0), av(1), av(2), av(3)]

            # ---------- softmax epilogue as filler chunks ----------
            def epi_e0(state):
                # evacuate psum (denom row 0, y rows 64:128) immediately
                # so the PV accumulator bank frees early
                b, qt, qcols, pvm = state
                yraw = epip.tile([P, 1024], f32, tag="yraw")
                nc.vector.tensor_copy(yraw[:, 0:512], pvm[:, 0:512])
                nc.scalar.copy(yraw[:, 512:1024], pvm[:, 512:1024])
                return yraw

            def epi_chunks(state, yraw):
                b, qt, qcols, pvm = state
                st_ = {"yraw": yraw}

                def e1():
                    dn = epip.tile([1, 1024], f32, tag="dn")
                    nc.vector.reciprocal_approx_fast(dn[:],
                                                     st_["yraw"][0:1, :])
                    st_["dn"] = dn

                def e2():
                    # broadcast 1/denom across partitions on the idle gpsimd
                    rep = epip.tile([P, 1024], f32, tag="rep")
                    nc.gpsimd.partition_broadcast(rep[:], st_["dn"][0:1, :])
                    st_["rep"] = rep

                def e3():
                    ynorm = epip.tile([P, 1024], bf16, tag="ynorm")
                    nc.vector.tensor_mul(ynorm[64:128, :],
                                         st_["yraw"][64:128, :],
                                         st_["rep"][64:128, :])
                    nc.sync.dma_start(yT[0:64, qcols], ynorm[64:128, 0:512])
                    nc.sync.dma_start(yT[64:128, qcols],
                                      ynorm[64:128, 512:1024])

                return [e1, e2, e3]

            # ---------- phase D (output projection) as filler chunks ------
            def d_chunk(rr, half):
                def d1():
                    oph = smp.tile([P, 512], f32, tag="sm",
                                   name=f"op{rr}_{half}")
                    nc.tensor.matmul(oph[:], yT[:, rr * P:(rr + 1) * P],
                                     wo_sb[:, half * 512:(half + 1) * 512],
                                     start=True, stop=True)
                    ob = obp.tile([P, 512], bf16, tag="ob")
                    if (rr + half) % 2 == 0:
                        nc.vector.tensor_copy(ob[:], oph[:])
                    else:
                        nc.scalar.copy(ob[:], oph[:])
                    nc.sync.dma_start(
                        out_ext[rr * P:(rr + 1) * P,
                                half * 512:(half + 1) * 512], ob[:])
                return d1

            # ---------- attention q-tile with fillers ----------
            def phase_c(b, qt, fillers):
                qcols = slice(b * S + qt * RB, b * S + (qt + 1) * RB)
                rtq = b * QT_PER_B + qt
                nks = qt * 4 + 4
                pvm = pvp.tile([P, 1024], f32, tag="acc",
                               name=f"pvm{b}_{qt}")
                pts = {}

                def j0_of(ks):
                    m = ks - qt * 4
                    return m * P if m >= 1 else 0

                def emit_pv(kk):
                    jj = j0_of(kk)
                    ptk = pts.pop(kk)
                    first, last = (kk == 0), (kk == nks - 1)
                    for h in range(2):
                        csl = slice(h * 512 + jj, (h + 1) * 512)
                        # v_aug = [ones | pad | v]: denominator lands on psum
                        # partition 0, y rows on 64:128, in ONE M=128 matmul
                        nc.tensor.matmul(
                            pvm[:, csl],
                            v_sb[:, b * (S // P) + kk, h],
                            ptk[:, h, jj:], start=first, stop=last)

                for ks in range(nks):
                    rtk = b * QT_PER_B + ks // 4
                    c0 = (ks % 4) * P
                    m = ks - qt * 4
                    j0 = j0_of(ks)
                    st = stp.tile([P, 1024], f32, tag="st",
                                  name=f"st{b}_{qt}_{ks}")
                    stv = st[:].rearrange("p (h c) -> p h c", h=2)
                    pt = ptp.tile([P, 2, RB], bf16, tag="pt")
                    pts[ks] = pt
                    for h in range(2):
                        hsl = slice(h * 64, (h + 1) * 64)
                        nc.tensor.matmul(
                            st[:, h * 512 + j0:(h + 1) * 512],
                            qkT[hsl, rtk, 1, c0:c0 + P],
                            qkT[hsl, rtq, 0, j0:],
                            start=True, stop=True)
                    nc.scalar.activation(pt[:, :, j0:], stv[:, :, j0:],
                                         mybir.ActivationFunctionType.Exp)
                    if m >= 0:
                        triv = tri_sb[:].rearrange("p (a c) -> p a c", a=2)
                        nc.vector.tensor_mul(pt[:, :, j0:j0 + P],
                                             pt[:, :, j0:j0 + P], triv)
                    for _ in range(2):
                        if fillers:
                            fillers.popleft()()
                    if ks >= 2:
                        emit_pv(ks - 2)
                for kk in (nks - 2, nks - 1):
                    emit_pv(kk)
                return (b, qt, qcols, pvm)

            # ---------- final-tile epilogue: half-pipelined with d ---------
            def epi_final(state, yraw):
                b, qt, qcols, pvm = state
                rr0 = (b * QT_PER_B + qt) * 4
                dn = epip.tile([1, 1024], f32, tag="dn")
                rep = epip.tile([P, 1024], f32, tag="rep")
                ynorm = epip.tile([P, 1024], bf16, tag="ynorm")
                q0 = qcols.start
                dpend = []
                for j in range(2):
                    sl0 = slice(j * 256, (j + 1) * 256)
                    sl1 = slice(512 + j * 256, 512 + (j + 1) * 256)
                    for sl in (sl0, sl1):
                        nc.vector.reciprocal_approx_fast(dn[:, sl],
                                                         yraw[0:1, sl])
                        nc.gpsimd.partition_broadcast(rep[:, sl],
                                                      dn[0:1, sl])
                        nc.vector.tensor_mul(ynorm[64:128, sl],
                                             yraw[64:128, sl],
                                             rep[64:128, sl])
                    nc.sync.dma_start(yT[0:64, q0 + j * 256:q0 + j * 256
                                         + 256], ynorm[64:128, sl0])
                    nc.sync.dma_start(yT[64:128, q0 + j * 256:q0 + j * 256
                                          + 256], ynorm[64:128, sl1])
                    for _ in range(2):
                        if dq:
                            dq.popleft()()
                    for dd in dpend:
                        dd()
                    dpend = [d_chunk(rr0 + 2 * j + t, hh)
                             for t in range(2) for hh in range(2)]
                for dd in dpend:
                    dd()
                while dq:
                    dq.popleft()()

            # ---------- master schedule ----------
            dq = deque()          # deferred output-projection half chunks
            for ch in a_chunks(0):
                ch()
            for ch in a_chunks(1):
                ch()
            prev = None
            prev_yraw = None
            tile_order = [(0, 0), (0, 1), (0, 2), (0, 3),
                          (1, 0), (1, 1), (1, 2), (1, 3)]
            for rt, (b, qt) in enumerate(tile_order):
                fillers = deque()
                if prev is not None:
                    fillers.extend(epi_chunks(prev, prev_yraw))
                    rrp = (prev[0] * QT_PER_B + prev[1]) * 4
                    dq.extend(d_chunk(rrp + t, hh)
                              for t in range(4) for hh in range(2))
                if rt < NRB - 2:
                    fillers.extend(a_chunks(rt + 2))
                nks = qt * 4 + 4
                cap = 2 * nks + 4 if rt < NRB - 1 else nks + 4
                while len(fillers) < cap and dq:
                    fillers.append(dq.popleft())
                prev = phase_c(b, qt, fillers)
                prev_yraw = epi_e0(prev)
                while fillers:
                    fillers.popleft()()
            epi_final(prev, prev_yraw)

    nc.finalize()
    return nc


def _host_inputs():
    t = np.arange(32, dtype=np.float64)
    inv_freq = 1.0 / (ROPE_BASE ** (2.0 * t / DH))
    pos = np.arange(S, dtype=np.float64)
    ang = pos[None, :] * inv_freq[:, None]          # [32, S]
    cos32 = np.cos(ang).astype(np.float32)
    sin32 = np.sin(ang).astype(np.float32)
    cc = np.tile(cos32, (4, 1))                     # [128, S]
    ss = np.concatenate([-sin32, sin32, -sin32, sin32], axis=0)  # [128, S]
    # layout [P, pos_tile(4), dup(2), 512] so per-rt slices are contiguous
    cc2 = np.repeat(cc.reshape(P, 4, 1, RB), 2, axis=2).reshape(P, -1)
    ss2 = np.repeat(ss.reshape(P, 4, 1, RB), 2, axis=2).reshape(P, -1)

    ii = np.arange(P)[:, None]
    uu = np.arange(P)[None, :]
    tri = (uu >= ii).astype(np.float32)             # [128, 128]
    tri2 = np.concatenate([tri, tri], axis=1)       # [128, 256]

    perm64 = np.concatenate([np.arange(0, 64, 2), np.arange(1, 64, 2)])
    return cc2, ss2, tri2, perm64


def _in_maps(x, Wq, bq, Wk, bk, Wv, bv, Wo):
    cc2, ss2, tri2, perm64 = _host_inputs()
    pswm = np.zeros((P, P), dtype=np.float32)
    for m_ in range(P):
        k_ = (m_ & ~63) | ((m_ + 32) & 63)
        pswm[k_, m_] = 1.0
    pswm = pswm.astype(nbf16)
    x2 = np.ascontiguousarray(x.reshape(NROWS, D))
    # xT block-major: xT[p, rt, d, c] = x[512*rt + c, 128*d + p]
    xT = np.ascontiguousarray(
        x2.reshape(NRB, RB, DSUB, P).transpose(3, 0, 2, 1)
        .reshape(P, NRB * DSUB * RB)).astype(nbf16)
    perm128 = np.concatenate([perm64, perm64 + 64])
    cc2b = cc2.astype(nbf16)
    ss2b = ss2.astype(nbf16)
    tri2b = tri2.astype(nbf16)
    def warr(wT):
        # [D, CH] -> [P, DSUB*CH]: w[p, d*CH+c] = wT[d*P+p, c]
        return np.ascontiguousarray(
            wT.reshape(DSUB, P, CH).transpose(1, 0, 2)
            .reshape(P, DSUB * CH)).astype(nbf16)

    maps = []
    for c in range(8):
        sl = slice(c * CH, (c + 1) * CH)
        maps.append({
            "xT": xT,
            "wqT": warr((Wq[sl][perm128] * 0.125).T),
            "wkT": warr(Wk[sl][perm128].T),
            "wvT": warr(Wv[sl].T),
            "woT": np.ascontiguousarray(Wo[:, sl].T).astype(nbf16),
            "bq": (bq[sl][perm128] * 0.125).reshape(CH, 1).copy(),
            "bk": bk[sl][perm128].reshape(CH, 1).copy(),
            "bvb": np.ascontiguousarray(
                np.tile(bv[sl].reshape(1, CH), (P, 1))),
            "cc2": cc2b, "ss2": ss2b, "tri": tri2b, "pswm": pswm,
        })
    return maps


def kernel(x, Wq, bq, Wk, bk, Wv, bv, Wo, bo):
    x = np.asarray(x, dtype=np.float32)
    Wq = np.asarray(Wq, dtype=np.float32)
    Wk = np.asarray(Wk, dtype=np.float32)
    Wv = np.asarray(Wv, dtype=np.float32)
    Wo = np.asarray(Wo, dtype=np.float32)
    bq = np.asarray(bq, dtype=np.float32)
    bk = np.asarray(bk, dtype=np.float32)
    bv = np.asarray(bv, dtype=np.float32)
    bo = np.asarray(bo, dtype=np.float32)

    if "nc" not in _CACHE:
        _CACHE["nc"] = _build()
    nc = _CACHE["nc"]

    res = run_bass_kernel_spmd(nc, _in_maps(x, Wq, bq, Wk, bk, Wv, bv, Wo),
                               core_ids=list(range(8)))
    out = np.zeros((NROWS, D), dtype=np.float32)
    for r in res.results:
        out += r["out"].astype(np.float32)
    out += bo[None, :]
    return out.reshape(B, S, D)


# revision 19
# speedup vs baseline: 1.0362x; 1.0103x over previous
"""Multi-head attention (B=2, S=2048, D=1024, H=16, causal, interleaved RoPE)
on 8 Trainium2 NeuronCores.

Sharding: tensor-parallel over heads - 2 heads (128 channels) per core.
Each core computes its Q/K/V projections, RoPE, causal attention, and a
row-parallel partial of the output projection; the host sums the bf16
partials in fp32.

All matmuls in bf16 with fp32 PSUM accumulation. Key structure:
  * x^T is pre-transposed and cast to bf16 on the host (block-major layout)
    so no on-device transposes are needed for the q/k projections.
  * Q/K projection weights are host-permuted so each head's dims are
    [evens(32), odds(32)]; the RoPE pair-swap is then a 32-partition-block
    permutation done with ONE PE matmul against a 0/1 permutation matrix.
  * V is projected directly in transposed layout (lhsT = x^T block, rhs =
    Wv^T) as N=128 matmuls, so v_sb[k-pos, ch] needs no PE transposes and
    the psum->sbuf evacuation is a single contiguous copy fused with the
    bias add.
  * Attention uses the S^T layout: scores psum [k(128part), q(512)] via
    matmul(lhsT=K^T, rhs=Q^T); the two heads run CONCURRENTLY on disjoint
    PE row groups (K=64 each, auto tile_position from base partition),
    writing the two banks of one [128,1024] psum tile; ONE exp over both
    heads; causal masking multiplies only the [128,2x128] diagonal strip
    by a triangular constant (DVE). PV via matmul(lhsT=V_aug, rhs=P^T)
    with V_aug = [ones | 63 zero-pad | v dims] per head: the softmax
    denominator lands on psum partition 0 and y-rows on partitions
    64-127 in one M=128 matmul (LDW stays hidden).
  * Softmax epilogue: one DVE + one ACT copy move [96,1024] (y + denom
    rows) out of psum immediately, releasing the PV accumulator two steps
    into the next tile; reciprocal/broadcast/normalize then run entirely
    off the critical path on SBUF data.
  * Fine-grained software pipelining: projection blocks, epilogue chunks
    and output-projection halves are emitted as filler bundles between the
    ks-steps of the attention loop so the PE queue always holds
    independent work (keeps HAM at 2.4 GHz).
"""

from collections import deque

import numpy as np
import ml_dtypes

import concourse.bacc as bacc
import concourse.mybir as mybir
import concourse.tile as tile
from concourse.bass_utils import run_bass_kernel_spmd

P = 128
B, S, D = 2, 2048, 1024
H, DH = 16, 64
NROWS = B * S            # 4096 flattened rows
CH = 128                 # channels per core (2 heads)
RB = 512                 # row block for projections / q tiles
NRB = NROWS // RB        # 8
DSUB = D // P            # 8 contraction subtiles
KSUB = NROWS // P        # 32 k subtiles (128 rows each)
QT_PER_B = S // RB       # 4 q tiles per batch
ROPE_BASE = 10000.0

f32 = mybir.dt.float32
bf16 = mybir.dt.bfloat16
nbf16 = ml_dtypes.bfloat16

_CACHE = {}


def _build():
    nc = bacc.Bacc("TRN2", target_bir_lowering=False)

    xT_ext = nc.declare_dram_parameter("xT", [P, NRB * DSUB * RB], bf16,
                                       isOutput=False)
    wqT_ext = nc.declare_dram_parameter("wqT", [P, DSUB * CH], bf16,
                                        isOutput=False)
    wkT_ext = nc.declare_dram_parameter("wkT", [P, DSUB * CH], bf16,
                                        isOutput=False)
    wvT_ext = nc.declare_dram_parameter("wvT", [P, DSUB * CH], bf16,
                                        isOutput=False)
    woT_ext = nc.declare_dram_parameter("woT", [CH, D], bf16, isOutput=False)
    bq_ext = nc.declare_dram_parameter("bq", [CH, 1], f32, isOutput=False)
    bk_ext = nc.declare_dram_parameter("bk", [CH, 1], f32, isOutput=False)
    bvb_ext = nc.declare_dram_parameter("bvb", [P, CH], f32, isOutput=False)
    cc_ext = nc.declare_dram_parameter("cc2", [P, QT_PER_B * 2 * RB], bf16,
                                       isOutput=False)
    ss_ext = nc.declare_dram_parameter("ss2", [P, QT_PER_B * 2 * RB], bf16,
                                       isOutput=False)
    tri_ext = nc.declare_dram_parameter("tri", [P, 2 * P], bf16,
                                        isOutput=False)
    psw_ext = nc.declare_dram_parameter("pswm", [P, P], bf16, isOutput=False)
    out_ext = nc.declare_dram_parameter("out", [NROWS, D], bf16, isOutput=True)

    with tile.TileContext(nc) as tc:
        with (
            tc.tile_pool(name="const", bufs=1) as cpool,
            tc.tile_pool(name="xpool", bufs=NRB) as xpool,
            tc.tile_pool(name="big", bufs=1) as big,
            tc.tile_pool(name="work", bufs=3) as work,
            tc.tile_pool(name="ptp", bufs=8) as ptp,
            tc.tile_pool(name="epi", bufs=2) as epip,
            tc.tile_pool(name="obp", bufs=4) as obp,
            tc.tile_pool(name="stp", bufs=2, space="PSUM") as stp,
            tc.tile_pool(name="pvp", bufs=1, space="PSUM") as pvp,
            tc.tile_pool(name="smp", bufs=2, space="PSUM") as smp,
        ):
            # ---- HAM warmup: dependency-free dummy matmuls run during the
            # input-DMA ramp so real compute starts at the 2.4 GHz clock ----
            wf = cpool.tile([P, P], f32, tag="warmf")
            nc.vector.memset(wf[:], 0.0)
            warm = cpool.tile([P, P], bf16, tag="warm")
            nc.vector.tensor_copy(warm[:], wf[:])
            wps = smp.tile([P, 512], f32, tag="sm", name="warmps")
            for _ in range(40):
                nc.tensor.matmul(wps[:, 0:P], warm[:], warm[:],
                                 start=True, stop=True)

            # ---- input DMAs, ordered so block 0's deps land first ----
            wq_sb = cpool.tile([P, DSUB, CH], bf16, tag="wq")
            nc.sync.dma_start(wq_sb[:, 0:2].rearrange("p d c -> p (d c)"),
                              wqT_ext[:, 0:2 * CH])
            nc.sync.dma_start(wq_sb[:, 2:4].rearrange("p d c -> p (d c)"),
                              wqT_ext[:, 2 * CH:4 * CH])
            xTb = []

            def load_xt(rt, split=1):
                xt = xpool.tile([P, DSUB, RB], bf16, tag="xT", name=f"xT{rt}")
                o = rt * DSUB * RB
                w = DSUB // split
                for j in range(split):
                    nc.sync.dma_start(
                        xt[:, j * w:(j + 1) * w]
                        .rearrange("p d c -> p (d c)"),
                        xT_ext[:, o + j * w * RB:o + (j + 1) * w * RB])
                xTb.append(xt)

            load_xt(0, split=8)
            nc.sync.dma_start(wq_sb[:, 4:8].rearrange("p d c -> p (d c)"),
                              wqT_ext[:, 4 * CH:8 * CH])
            wk_sb = cpool.tile([P, DSUB, CH], bf16, tag="wk")
            nc.sync.dma_start(wk_sb[:].rearrange("p d c -> p (d c)"),
                              wkT_ext[:])
            bq_sb = cpool.tile([CH, 1], f32, tag="bq")
            nc.sync.dma_start(bq_sb[:], bq_ext[:])
            psw_sb = cpool.tile([P, P], bf16, tag="pswm")
            nc.sync.dma_start(psw_sb[:], psw_ext[:])
            bk_sb = cpool.tile([CH, 1], f32, tag="bk")
            nc.sync.dma_start(bk_sb[:], bk_ext[:])
            wv_sb = cpool.tile([P, DSUB, CH], bf16, tag="wv")
            nc.sync.dma_start(wv_sb[:].rearrange("p d c -> p (d c)"),
                              wvT_ext[:])
            bvb_sb = cpool.tile([P, CH], f32, tag="bvb")
            nc.sync.dma_start(bvb_sb[:], bvb_ext[:])
            cc_sb = cpool.tile([P, QT_PER_B, 2, RB], bf16, tag="cc")
            ss_sb = cpool.tile([P, QT_PER_B, 2, RB], bf16, tag="ss")
            for j in range(2):
                csl = slice(j * 4 * RB, (j + 1) * 4 * RB)
                nc.sync.dma_start(
                    cc_sb[:, 2 * j:2 * j + 2]
                    .rearrange("p a b c -> p (a b c)"), cc_ext[:, csl])
                nc.sync.dma_start(
                    ss_sb[:, 2 * j:2 * j + 2]
                    .rearrange("p a b c -> p (a b c)"), ss_ext[:, csl])
            tri_sb = cpool.tile([P, 2 * P], bf16, tag="tri")
            nc.sync.dma_start(tri_sb[:], tri_ext[:])
            load_xt(1, split=2)
            for rt in range(2, NRB):
                load_xt(rt)
            wo_sb = cpool.tile([CH, D], bf16, tag="wo")
            nc.sync.dma_start(wo_sb[:, 0:512], woT_ext[:, 0:512])
            nc.sync.dma_start(wo_sb[:, 512:1024], woT_ext[:, 512:1024])

            # ---- constants ----
            ones_f = cpool.tile([P, 2 * KSUB], f32, tag="onesf")
            nc.vector.memset(ones_f[:], 1.0)

            # ---- persistent activation tiles ----
            qkT = big.tile([P, NRB, 2, RB], bf16, tag="qkT")
            yT = big.tile([P, NROWS], bf16, tag="yT")
            v_sb = big.tile([P, KSUB, 2, CH], bf16, tag="v")
            # per head: [ones | 63 pad | 64 v-dims] = 128 cols
            nc.vector.tensor_copy(
                v_sb[:, :, :, 0:1].rearrange("p a b c -> p (a b c)"),
                ones_f[:])
            nc.vector.memset(v_sb[:, :, :, 1:64], 0.0)

            # ---------- phase A (projections + RoPE) as filler chunks ------
            def a_chunks(rt):
                pos = rt % QT_PER_B
                xt = xTb[rt]
                st_ = {}

                def a1():
                    pq = smp.tile([P, RB], f32, tag="sm", name=f"pq{rt}")
                    st_["pq"] = pq
                    for d in range(4):
                        nc.tensor.matmul(pq[:], wq_sb[:, d], xt[:, d],
                                         start=(d == 0), stop=False)

                def a2():
                    pq = st_["pq"]
                    for d in range(4, 8):
                        nc.tensor.matmul(pq[:], wq_sb[:, d], xt[:, d],
                                         start=False, stop=(d == 7))
                    praw = work.tile([P, 2, RB], bf16, tag="praw")
                    st_["praw"] = praw
                    nc.vector.tensor_scalar_add(praw[:, 0], pq[:],
                                                bq_sb[:, 0:1])

                def a3():
                    pk = smp.tile([P, RB], f32, tag="sm", name=f"pk{rt}")
                    st_["pk"] = pk
                    for d in range(4):
                        nc.tensor.matmul(pk[:], wk_sb[:, d], xt[:, d],
                                         start=(d == 0), stop=False)

                def a4():
                    pk = st_["pk"]
                    for d in range(4, 8):
                        nc.tensor.matmul(pk[:], wk_sb[:, d], xt[:, d],
                                         start=False, stop=(d == 7))
                    nc.vector.tensor_scalar_add(st_["praw"][:, 1], pk[:],
                                                bk_sb[:, 0:1])

                def a5():
                    # RoPE pair swap via PE permutation matmul
                    praw = st_["praw"]
                    swq = smp.tile([P, RB], f32, tag="sm", name=f"swq{rt}")
                    swk = smp.tile([P, RB], f32, tag="sm", name=f"swk{rt}")
                    st_["swq"], st_["swk"] = swq, swk
                    nc.tensor.matmul(swq[:], psw_sb[:], praw[:, 0],
                                     start=True, stop=True)
                    nc.tensor.matmul(swk[:], psw_sb[:], praw[:, 1],
                                     start=True, stop=True)

                def a6():
                    praw = st_["praw"]
                    t1 = work.tile([P, 2, RB], bf16, tag="t1")
                    st_["t1"] = t1
                    nc.vector.tensor_mul(t1[:], praw[:], cc_sb[:, pos])
                    t2q = work.tile([P, RB], bf16, tag="t2q")
                    nc.vector.tensor_mul(t2q[:], st_["swq"][:],
                                         ss_sb[:, pos, 0])
                    nc.vector.tensor_add(qkT[:, rt, 0], t1[:, 0], t2q[:])

                def a7():
                    t2k = work.tile([P, RB], bf16, tag="t2k")
                    nc.vector.tensor_mul(t2k[:], st_["swk"][:],
                                         ss_sb[:, pos, 1])
                    nc.vector.tensor_add(qkT[:, rt, 1], st_["t1"][:, 1],
                                         t2k[:])

                def av(blk):
                    def f():
                        vps = smp.tile([P, CH], f32, tag="sm",
                                       name=f"vps{rt}_{blk}")
                        for d in range(DSUB):
                            nc.tensor.matmul(
                                vps[:], xt[:, d, blk * P:(blk + 1) * P],
                                wv_sb[:, d], start=(d == 0), stop=(d == 7))
                        nc.vector.tensor_add(
                            v_sb[:, rt * 4 + blk, :, 64:128],
                            vps[:].rearrange("p (h c) -> p h c", h=2),
                            bvb_sb[:].rearrange("p (h c) -> p h c", h=2))
                    return f

                return [a1, a2, a3, a4, a5, a6, a7,
                        av(0), av(1), av(2), av(3)]

            # ---------- softmax epilogue as filler chunks ----------
            def epi_e0(state):
                # evacuate psum (denom row 0, y rows 64:128) immediately
                # so the PV accumulator bank frees early
                b, qt, qcols, pvm = state
                yraw = epip.tile([P, 1024], f32, tag="yraw")
                nc.vector.tensor_copy(yraw[:, 0:512], pvm[:, 0:512])
                nc.scalar.copy(yraw[:, 512:1024], pvm[:, 512:1024])
                return yraw

            def epi_chunks(state, yraw):
                b, qt, qcols, pvm = state
                st_ = {"yraw": yraw}

                def e1():
                    dn = epip.tile([1, 1024], f32, tag="dn")
                    nc.vector.reciprocal_approx_fast(dn[:],
                                                     st_["yraw"][0:1, :])
                    st_["dn"] = dn

                def e2():
                    # broadcast 1/denom across partitions on the idle gpsimd
                    rep = epip.tile([P, 1024], f32, tag="rep")
                    nc.gpsimd.partition_broadcast(rep[:], st_["dn"][0:1, :])
                    st_["rep"] = rep

                def e3():
                    ynorm = epip.tile([P, 1024], bf16, tag="ynorm")
                    nc.vector.tensor_mul(ynorm[64:128, :],
                                         st_["yraw"][64:128, :],
                                         st_["rep"][64:128, :])
                    nc.sync.dma_start(yT[0:64, qcols], ynorm[64:128, 0:512])
                    nc.sync.dma_start(yT[64:128, qcols],
                                      ynorm[64:128, 512:1024])

                return [e1, e2, e3]

            # ---------- phase D (output projection) as filler chunks ------
            def d_chunk(rr, half):
                def d1():
                    oph = smp.tile([P, 512], f32, tag="sm",
                                   name=f"op{rr}_{half}")
                    nc.tensor.matmul(oph[:], yT[:, rr * P:(rr + 1) * P],
                                     wo_sb[:, half * 512:(half + 1) * 512],
                                     start=True, stop=True)
                    ob = obp.tile([P, 512], bf16, tag="ob")
                    if (rr + half) % 2 == 0:
                        nc.vector.tensor_copy(ob[:], oph[:])
                    else:
                        nc.scalar.copy(ob[:], oph[:])
                    nc.sync.dma_start(
                        out_ext[rr * P:(rr + 1) * P,
                                half * 512:(half + 1) * 512], ob[:])
                return d1

            # ---------- attention q-tile with fillers ----------
            def phase_c(b, qt, fillers):
                qcols = slice(b * S + qt * RB, b * S + (qt + 1) * RB)
                rtq = b * QT_PER_B + qt
                nks = qt * 4 + 4
                pvm = pvp.tile([P, 1024], f32, tag="acc",
                               name=f"pvm{b}_{qt}")
                pts = {}

                def j0_of(ks):
                    m = ks - qt * 4
                    return m * P if m >= 1 else 0

                def emit_pv(kk):
                    jj = j0_of(kk)
                    ptk = pts.pop(kk)
                    first, last = (kk == 0), (kk == nks - 1)
                    for h in range(2):
                        csl = slice(h * 512 + jj, (h + 1) * 512)
                        # v_aug = [ones | pad | v]: denominator lands on psum
                        # partition 0, y rows on 64:128, in ONE M=128 matmul
                        nc.tensor.matmul(
                            pvm[:, csl],
                            v_sb[:, b * (S // P) + kk, h],
                            ptk[:, h, jj:], start=first, stop=last)

                for ks in range(nks):
                    rtk = b * QT_PER_B + ks // 4
                    c0 = (ks % 4) * P
                    m = ks - qt * 4
                    j0 = j0_of(ks)
                    st = stp.tile([P, 1024], f32, tag="st",
                                  name=f"st{b}_{qt}_{ks}")
                    stv = st[:].rearrange("p (h c) -> p h c", h=2)
                    pt = ptp.tile([P, 2, RB], bf16, tag="pt")
                    pts[ks] = pt
                    for h in range(2):
                        hsl = slice(h * 64, (h + 1) * 64)
                        nc.tensor.matmul(
                            st[:, h * 512 + j0:(h + 1) * 512],
                            qkT[hsl, rtk, 1, c0:c0 + P],
                            qkT[hsl, rtq, 0, j0:],
                            start=True, stop=True)
                    nc.scalar.activation(pt[:, :, j0:], stv[:, :, j0:],
                                         mybir.ActivationFunctionType.Exp)
                    if m >= 0:
                        triv = tri_sb[:].rearrange("p (a c) -> p a c", a=2)
                        nc.vector.tensor_mul(pt[:, :, j0:j0 + P],
                                             pt[:, :, j0:j0 + P], triv)
                    for _ in range(2):
                        if fillers:
                            fillers.popleft()()
                    if ks >= 2:
                        emit_pv(ks - 2)
                for kk in (nks - 2, nks - 1):
                    emit_pv(kk)
                return (b, qt, qcols, pvm)

            # ---------- final-tile epilogue: half-pipelined with d ---------
            def epi_final(state, yraw):
                b, qt, qcols, pvm = state
                rr0 = (b * QT_PER_B + qt) * 4
                dn = epip.tile([1, 1024], f32, tag="dn")
                rep = epip.tile([P, 1024], f32, tag="rep")
                ynorm = epip.tile([P, 1024], bf16, tag="ynorm")
                q0 = qcols.start
                dpend = []
                for j in range(2):
                    sl0 = slice(j * 256, (j + 1) * 256)
                    sl1 = slice(512 + j * 256, 512 + (j + 1) * 256)
                    for sl in (sl0, sl1):
                        nc.vector.reciprocal_approx_fast(dn[:, sl],
                                                         yraw[0:1, sl])
                        nc.gpsimd.partition_broadcast(rep[:, sl],
                                                      dn[0:1, sl])
                        nc.vector.tensor_mul(ynorm[64:128, sl],
                                             yraw[64:128, sl],
                                             rep[64:128, sl])
                    nc.sync.dma_start(yT[0:64, q0 + j * 256:q0 + j * 256
                                         + 256], ynorm[64:128, sl0])
                    nc.sync.dma_start(yT[64:128, q0 + j * 256:q0 + j * 256
                                          + 256], ynorm[64:128, sl1])
                    for _ in range(2):
                        if dq:
                            dq.popleft()()
                    for dd in dpend:
                        dd()
                    dpend = [d_chunk(rr0 + 2 * j + t, hh)
                             for t in range(2) for hh in range(2)]
                for dd in dpend:
                    dd()
                while dq:
                    dq.popleft()()

            # ---------- master schedule ----------
            dq = deque()          # deferred output-projection half chunks
            for ch in a_chunks(0):
                ch()
            for ch in a_chunks(1):
                ch()
            prev = None
            prev_yraw = None
            tile_order = [(0, 0), (0, 1), (0, 2), (0, 3),
                          (1, 0), (1, 1), (1, 2), (1, 3)]
            for rt, (b, qt) in enumerate(tile_order):
                fillers = deque()
                if prev is not None:
                    fillers.extend(epi_chunks(prev, prev_yraw))
                    rrp = (prev[0] * QT_PER_B + prev[1]) * 4
                    dq.extend(d_chunk(rrp + t, hh)
                              for t in range(4) for hh in range(2))
                if rt < NRB - 2:
                    fillers.extend(a_chunks(rt + 2))
                nks = qt * 4 + 4
                cap = 2 * nks + 4 if rt < NRB - 1 else nks + 4
                keep = 4 if rt == NRB - 1 else 0
                while len(fillers) < cap and len(dq) > keep:
                    fillers.append(dq.popleft())
                prev = phase_c(b, qt, fillers)
                prev_yraw = epi_e0(prev)
                while fillers:
                    fillers.popleft()()
            epi_final(prev, prev_yraw)

    nc.finalize()
    return nc


def _host_inputs():
    t = np.arange(32, dtype=np.float64)
    inv_freq = 1.0 / (ROPE_BASE ** (2.0 * t / DH))
    pos = np.arange(S, dtype=np.float64)
    ang = pos[None, :] * inv_freq[:, None]          # [32, S]
    cos32 = np.cos(ang).astype(np.float32)
    sin32 = np.sin(ang).astype(np.float32)
    cc = np.tile(cos32, (4, 1))                     # [128, S]
    ss = np.concatenate([-sin32, sin32, -sin32, sin32], axis=0)  # [128, S]
    # layout [P, pos_tile(4), dup(2), 512] so per-rt slices are contiguous
    cc2 = np.repeat(cc.reshape(P, 4, 1, RB), 2, axis=2).reshape(P, -1)
    ss2 = np.repeat(ss.reshape(P, 4, 1, RB), 2, axis=2).reshape(P, -1)

    ii = np.arange(P)[:, None]
    uu = np.arange(P)[None, :]
    tri = (uu >= ii).astype(np.float32)             # [128, 128]
    tri2 = np.concatenate([tri, tri], axis=1)       # [128, 256]

    perm64 = np.concatenate([np.arange(0, 64, 2), np.arange(1, 64, 2)])
    return cc2, ss2, tri2, perm64


def _in_maps(x, Wq, bq, Wk, bk, Wv, bv, Wo):
    cc2, ss2, tri2, perm64 = _host_inputs()
    pswm = np.zeros((P, P), dtype=np.float32)
    for m_ in range(P):
        k_ = (m_ & ~63) | ((m_ + 32) & 63)
        pswm[k_, m_] = 1.0
    pswm = pswm.astype(nbf16)
    x2 = np.ascontiguousarray(x.reshape(NROWS, D))
    # xT block-major: xT[p, rt, d, c] = x[512*rt + c, 128*d + p]
    xT = np.ascontiguousarray(
        x2.reshape(NRB, RB, DSUB, P).transpose(3, 0, 2, 1)
        .reshape(P, NRB * DSUB * RB)).astype(nbf16)
    perm128 = np.concatenate([perm64, perm64 + 64])
    cc2b = cc2.astype(nbf16)
    ss2b = ss2.astype(nbf16)
    tri2b = tri2.astype(nbf16)
    def warr(wT):
        # [D, CH] -> [P, DSUB*CH]: w[p, d*CH+c] = wT[d*P+p, c]
        return np.ascontiguousarray(
            wT.reshape(DSUB, P, CH).transpose(1, 0, 2)
            .reshape(P, DSUB * CH)).astype(nbf16)

    maps = []
    for c in range(8):
        sl = slice(c * CH, (c + 1) * CH)
        maps.append({
            "xT": xT,
            "wqT": warr((Wq[sl][perm128] * 0.125).T),
            "wkT": warr(Wk[sl][perm128].T),
            "wvT": warr(Wv[sl].T),
            "woT": np.ascontiguousarray(Wo[:, sl].T).astype(nbf16),
            "bq": (bq[sl][perm128] * 0.125).reshape(CH, 1).copy(),
            "bk": bk[sl][perm128].reshape(CH, 1).copy(),
            "bvb": np.ascontiguousarray(
                np.tile(bv[sl].reshape(1, CH), (P, 1))),
            "cc2": cc2b, "ss2": ss2b, "tri": tri2b, "pswm": pswm,
        })
    return maps


def kernel(x, Wq, bq, Wk, bk, Wv, bv, Wo, bo):
    x = np.asarray(x, dtype=np.float32)
    Wq = np.asarray(Wq, dtype=np.float32)
    Wk = np.asarray(Wk, dtype=np.float32)
    Wv = np.asarray(Wv, dtype=np.float32)
    Wo = np.asarray(Wo, dtype=np.float32)
    bq = np.asarray(bq, dtype=np.float32)
    bk = np.asarray(bk, dtype=np.float32)
    bv = np.asarray(bv, dtype=np.float32)
    bo = np.asarray(bo, dtype=np.float32)

    if "nc" not in _CACHE:
        _CACHE["nc"] = _build()
    nc = _CACHE["nc"]

    res = run_bass_kernel_spmd(nc, _in_maps(x, Wq, bq, Wk, bk, Wv, bv, Wo),
                               core_ids=list(range(8)))
    out = np.zeros((NROWS, D), dtype=np.float32)
    for r in res.results:
        out += r["out"].astype(np.float32)
    out += bo[None, :]
    return out.reshape(B, S, D)
